# revision 12
# baseline (speedup 1.0000x reference)
"""Trainium2 Bass kernel for nn_DeepSet_TM (DeepSet encode MLP -> per-feature
trimmed mean over ragged N -> decode MLP).

Strategy:
  - Data-parallel over B: 8 samples per core on 8 cores, identical SPMD
    program. Samples are sorted by valid length L and dealt round-robin to
    cores, so slot s on every core has a similar L; the slot's free-dim FD
    (max L in slot, rounded up to 128) is baked into the program, which cuts
    all matmul/scan work by ~25% on average. Per-sample scalars arrive as
    input tensors, so one compiled program serves any mask with the same
    slot-FD signature.
  - Matmuls in float32r (TF32-grade, 1 cyc/row) with transposed layouts so the
    per-feature reduction axis (n) lands on the free dimension.
  - Trimmed mean without sorting, via the CVaR duality
        sum_of_k_largest = min_t [ sum relu(x - t) + k t ],
    which is flat (2nd order) around the optimal t. t is initialized from
    per-feature mean/std (Gaussian quantile) and refined with one Newton step
    on exact counts: count_gt on VectorE (is_gt + accumulate), count_lt on
    ScalarE (Sign + accumulate). Final tails: S_hi on ScalarE (Relu +
    accumulate), S_lo on VectorE via S_lo(t) = L*t - sum_valid min(x, t).
  - Ragged handling: X pad rows are zeroed on host, so padded e-columns equal
    a per-feature constant; its exact (f32r) value is read back from the last
    e-column on device and used for exact pad corrections of all stats.
  - Software pipelining: encode(s) is emitted before select(s-1) so the PE and
    the evacuation engines stay ahead of the selection passes.
"""
import numpy as np

import concourse.bacc as bacc
import concourse.mybir as mybir
from concourse import masks
from concourse.tile import TileContext
from concourse.bass_utils import run_bass_kernel_spmd

B, N, D_IN, D_H, NOUT = 64, 1024, 512, 1024, 10
TRIM_RATIO = 0.1
NCORES = 8
SPC = B // NCORES          # samples (slots) per core
CH = 512                   # max n-chunk (f32 matmul moving-operand limit)
DT = D_IN // 128           # 4  d-tiles
HT = D_H // 128            # 8  h/f-tiles
SUBN = 256                 # always-valid prefix used for the variance estimate
F32 = mybir.dt.float32
F32R = mybir.dt.float32r
AF = mybir.ActivationFunctionType
ALU = mybir.AluOpType

# CONST columns (per-sample scalars, replicated over partitions)
C_INVL, C_Z, C_K, C_INVA, C_INVDEN, C_PADC, C_L = 0, 1, 2, 3, 4, 5, 6
NCC = 7


def _norm_ppf(p):
    """Acklam's rational approximation of the standard normal inverse CDF."""
    a = [-3.969683028665376e+01, 2.209460984245205e+02, -2.759285104469687e+02,
         1.383577518672690e+02, -3.066479806614716e+01, 2.506628277459239e+00]
    b = [-5.447609879822406e+01, 1.615858368580409e+02, -1.556989798598866e+02,
         6.680131188771972e+01, -1.328068155288572e+01]
    c = [-7.784894002430293e-03, -3.223964580411365e-01, -2.400758277161838e+00,
         -2.549732539343734e+00, 4.374664141464968e+00, 2.938163982698783e+00]
    d = [7.784695709041462e-03, 3.224671290700398e-01, 2.445134137142996e+00,
         3.754408661907416e+00]
    p = float(p)
    if p < 0.02425:
        q = np.sqrt(-2 * np.log(p))
        return (((((c[0]*q+c[1])*q+c[2])*q+c[3])*q+c[4])*q+c[5]) / \
               ((((d[0]*q+d[1])*q+d[2])*q+d[3])*q+1)
    if p > 1 - 0.02425:
        return -_norm_ppf(1 - p)
    q = p - 0.5
    r = q * q
    return (((((a[0]*r+a[1])*r+a[2])*r+a[3])*r+a[4])*r+a[5])*q / \
           (((((b[0]*r+b[1])*r+b[2])*r+b[3])*r+b[4])*r+1)


def _chunks(fd):
    out = [CH] * (fd // CH)
    if fd % CH:
        out.append(fd % CH)
    return out


_BUILD_CACHE = {}
_TRACE = False


def _build_program(fds):
    if fds in _BUILD_CACHE:
        return _BUILD_CACHE[fds]
    nc = bacc.Bacc("TRN2", target_bir_lowering=False, debug=False)

    X = nc.declare_dram_parameter("X", [SPC, N, D_IN], F32R, isOutput=False)
    W1 = nc.declare_dram_parameter("W1", [D_IN, D_H], F32R, isOutput=False)
    W2 = nc.declare_dram_parameter("W2", [D_H, D_H], F32R, isOutput=False)
    W3 = nc.declare_dram_parameter("W3", [D_H, D_H], F32R, isOutput=False)
    W4 = nc.declare_dram_parameter("W4", [D_H, NOUT], F32R, isOutput=False)
    B1 = nc.declare_dram_parameter("B1", [128, HT], F32, isOutput=False)
    B2 = nc.declare_dram_parameter("B2", [128, HT], F32, isOutput=False)
    B3 = nc.declare_dram_parameter("B3", [128, HT], F32, isOutput=False)
    B4 = nc.declare_dram_parameter("B4", [NOUT, 1], F32, isOutput=False)
    CONST = nc.declare_dram_parameter("CONST", [SPC, 128, NCC], F32,
                                      isOutput=False)
    Y = nc.declare_dram_parameter("Y", [NOUT, SPC], F32, isOutput=True)

    with TileContext(nc) as tc:
        with (
            tc.tile_pool(name="const", bufs=1) as pc,
            tc.tile_pool(name="xraw", bufs=2) as pxr,
            tc.tile_pool(name="xt", bufs=1) as pxt,
            tc.tile_pool(name="h1", bufs=2) as ph1,
            tc.tile_pool(name="epool", bufs=2) as pe,
            tc.tile_pool(name="scr", bufs=2) as pscr,
            tc.tile_pool(name="stats", bufs=2) as pst,
            tc.tile_pool(name="wstr", bufs=2) as pws,
            tc.tile_pool(name="ps_tp", bufs=2, space="PSUM") as ps_tp,
            tc.tile_pool(name="ps_h", bufs=2, space="PSUM") as ps_h,
            tc.tile_pool(name="ps_e", bufs=2, space="PSUM") as ps_e,
        ):
            # ---- first X chunk before the weights: un-gates the transposes --
            xr0 = pxr.tile([128, DT * CH], F32R, tag="xr")
            nc.sync.dma_start(
                out=xr0.rearrange("p (t d) -> p t d", t=DT),
                in_=X[0, 0:CH, :].rearrange("(t p) d -> p t d", p=128))

            # ---- resident constants / weights -------------------------------
            w1 = pc.tile([128, DT * D_H], F32R, tag="w1")
            nc.sync.dma_start(out=w1.rearrange("p (t h) -> p t h", t=DT),
                              in_=W1.rearrange("(t p) h -> p t h", p=128))
            w2 = pc.tile([128, HT * D_H], F32R, tag="w2")
            nc.sync.dma_start(out=w2.rearrange("p (t h) -> p t h", t=HT),
                              in_=W2.rearrange("(t p) h -> p t h", p=128))
            w4 = pc.tile([128, HT * NOUT], F32R, tag="w4")
            nc.sync.dma_start(out=w4.rearrange("p (t o) -> p t o", t=HT),
                              in_=W4.rearrange("(t p) o -> p t o", p=128))
            b1t = pc.tile([128, HT], F32, tag="b1t")
            nc.sync.dma_start(out=b1t[:], in_=B1[:])
            b2t = pc.tile([128, HT], F32, tag="b2t")
            nc.sync.dma_start(out=b2t[:], in_=B2[:])
            b3t = pc.tile([128, HT], F32, tag="b3t")
            nc.sync.dma_start(out=b3t[:], in_=B3[:])
            b4t = pc.tile([NOUT, 1], F32, tag="b4t")
            nc.sync.dma_start(out=b4t[:], in_=B4[:])
            ident = pc.tile([128, 128], F32, tag="ident")
            masks.make_identity(nc, ident[:])
            trimmed = pc.tile([128, SPC * HT], F32R, tag="trimmed")
            h3sb = pc.tile([128, HT * SPC], F32R, tag="h3sb")

            _stn = [0]

            def st(tag, cols=HT):
                _stn[0] += 1
                return pst.tile([128, cols], F32, tag=tag,
                                name=f"st_{tag}_{_stn[0]}")

            def scrtile(eng):
                _stn[0] += 1
                return pscr.tile([128, N], F32, tag=f"scr_{eng}",
                                 name=f"scr_{eng}_{_stn[0]}")

            # ================= encode stage ==================================
            def emit_encode(s):
                fd = fds[s]
                chs = _chunks(fd)
                cst = pst.tile([128, NCC], F32, tag="cst", name=f"cst{s}")
                nc.sync.dma_start(out=cst[:], in_=CONST[s])
                esb = [pe.tile([128, fd], F32, tag=f"e{ft}",
                               name=f"esb{s}_{ft}") for ft in range(HT)]
                sumxc = st("sumxc", HT)
                n0 = 0
                h1s = []
                for c, cfd in enumerate(chs):
                    ntt = cfd // 128
                    if s == 0 and c == 0:
                        xr = xr0
                    else:
                        xr = pxr.tile([128, DT * CH], F32R, tag="xr",
                                      name=f"xr{s}_{c}")
                        nc.sync.dma_start(
                            out=xr.rearrange("p (t d) -> p t d", t=DT)
                                [:, 0:ntt, :],
                            in_=X[s, n0:n0 + cfd, :]
                                .rearrange("(t p) d -> p t d", p=128))
                    xt = pxt.tile([128, DT * CH], F32R, tag="xt",
                                  name=f"xt{s}_{c}")
                    for dt in range(DT):
                        tp = ps_tp.tile([128, CH], F32, tag="tp",
                                        name=f"tp{s}_{c}_{dt}")
                        for nt in range(ntt):
                            nc.tensor.transpose(
                                tp[:, nt * 128:(nt + 1) * 128],
                                xr[:, nt * D_IN + dt * 128:
                                      nt * D_IN + (dt + 1) * 128].bitcast(F32),
                                ident[:])
                        nc.vector.tensor_copy(
                            xt[:, dt * CH:dt * CH + cfd], tp[:, 0:cfd])
                    h1 = ph1.tile([128, HT * CH], F32R, tag="h1",
                                  name=f"h1_{s}_{c}")
                    for ht in range(HT):
                        hp = ps_h.tile([128, CH], F32, tag="hp",
                                       name=f"hp{s}_{c}_{ht}")
                        for dt in range(DT):
                            nc.tensor.matmul(
                                hp[:, 0:cfd],
                                w1[:, dt * D_H + ht * 128:
                                      dt * D_H + (ht + 1) * 128],
                                xt[:, dt * CH:dt * CH + cfd],
                                start=(dt == 0), stop=(dt == DT - 1))
                        nc.vector.tensor_scalar(
                            out=h1[:, ht * CH:ht * CH + cfd], in0=hp[:, 0:cfd],
                            scalar1=b1t[:, ht:ht + 1], scalar2=0.0,
                            op0=ALU.add, op1=ALU.max)
                    h1s.append((h1, n0, cfd))
                    n0 += cfd
                for ft in range(HT):
                    ep = ps_e.tile([128, fd], F32, tag="ep",
                                   name=f"ep{s}_{ft}")
                    for h1, n0c, cfd in h1s:
                        for ht in range(HT):
                            nc.tensor.matmul(
                                ep[:, n0c:n0c + cfd],
                                w2[:, ht * D_H + ft * 128:
                                      ht * D_H + (ft + 1) * 128],
                                h1[:, ht * CH:ht * CH + cfd],
                                start=(ht == 0), stop=(ht == HT - 1))
                    nc.scalar.activation(
                        esb[ft][:], ep[:], AF.Identity,
                        bias=b2t[:, ft:ft + 1], scale=1.0,
                        accum_out=sumxc[:, ft:ft + 1])
                return cst, esb, sumxc, len(chs)

            # ================= selection stage ===============================
            def emit_select(s, cst, esb, sumxc, nch):
                fd = float(fds[s])
                fdi = fds[s]
                col = lambda j: cst[:, j:j + 1]
                # exact pad value: last e column (a pad whenever padc > 0)
                pdev = st("pdev")
                for ft in range(HT):
                    nc.gpsimd.tensor_copy(pdev[:, ft:ft + 1],
                                          esb[ft][:, fdi - 1:fdi])
                padp = st("padp")
                nc.gpsimd.tensor_scalar(out=padp[:], in0=pdev[:],
                                        scalar1=col(C_PADC), scalar2=None,
                                        op0=ALU.mult)
                sumx = st("sumx")
                nc.gpsimd.tensor_tensor(out=sumx[:], in0=sumxc[:],
                                        in1=padp[:], op=ALU.subtract)
                # variance from the always-valid first SUBN columns
                sumxx = st("sumxx")
                for ft in range(HT):
                    scr = scrtile("a")
                    nc.scalar.activation(scr[:, 0:SUBN], esb[ft][:, 0:SUBN],
                                         AF.Square,
                                         accum_out=sumxx[:, ft:ft + 1])
                mu = st("mu")
                nc.gpsimd.tensor_scalar(out=mu[:], in0=sumx[:],
                                        scalar1=col(C_INVL), scalar2=None,
                                        op0=ALU.mult)
                var = st("var")
                nc.gpsimd.tensor_scalar(out=var[:], in0=sumxx[:],
                                        scalar1=1.0 / SUBN, scalar2=None,
                                        op0=ALU.mult)
                musq = st("musq")
                nc.gpsimd.tensor_tensor(out=musq[:], in0=mu[:], in1=mu[:],
                                        op=ALU.mult)
                nc.gpsimd.tensor_tensor(out=var[:], in0=var[:], in1=musq[:],
                                        op=ALU.subtract)
                nc.gpsimd.tensor_scalar(out=var[:], in0=var[:], scalar1=1e-12,
                                        scalar2=None, op0=ALU.max)
                sig = st("sig")
                nc.scalar.activation(sig[:], var[:], AF.Sqrt)
                sigz = st("sigz")
                nc.gpsimd.tensor_scalar(out=sigz[:], in0=sig[:],
                                        scalar1=col(C_Z), scalar2=None,
                                        op0=ALU.mult)
                t0hi = st("t0hi")
                nc.gpsimd.tensor_tensor(out=t0hi[:], in0=mu[:], in1=sigz[:],
                                        op=ALU.add)
                t0lo = st("t0lo")
                nc.gpsimd.tensor_tensor(out=t0lo[:], in0=mu[:], in1=sigz[:],
                                        op=ALU.subtract)
                negt0lo = st("negt0lo")
                nc.gpsimd.tensor_scalar(out=negt0lo[:], in0=t0lo[:],
                                        scalar1=-1.0, scalar2=None,
                                        op0=ALU.mult)

                # counts at t0: c_gt on DVE (is_gt), c_lt on ACT (Sign)
                cgt = st("cgt")
                sgn = st("sgn")
                for ft in range(HT):
                    scr = scrtile("d")
                    nc.vector.tensor_scalar(
                        out=scr[:, 0:fdi], in0=esb[ft][:],
                        scalar1=t0hi[:, ft:ft + 1],
                        scalar2=0.0, op0=ALU.is_gt, op1=ALU.add,
                        accum_out=cgt[:, ft:ft + 1])
                    scr2 = scrtile("a")
                    nc.scalar.activation(
                        scr2[:, 0:fdi], esb[ft][:], AF.Sign,
                        bias=negt0lo[:, ft:ft + 1], scale=1.0,
                        accum_out=sgn[:, ft:ft + 1])
                # c_lt_all = (FD - sgn_sum)/2
                clt = st("clt")
                nc.gpsimd.tensor_scalar(out=clt[:], in0=sgn[:],
                                        scalar1=-0.5, scalar2=fd / 2.0,
                                        op0=ALU.mult, op1=ALU.add)

                def pad_count_fix(cnt, thr, op):
                    tmp = st("tmpa")
                    nc.vector.tensor_tensor(out=tmp[:], in0=pdev[:], in1=thr[:],
                                            op=op)
                    nc.gpsimd.tensor_scalar(out=tmp[:], in0=tmp[:],
                                            scalar1=col(C_PADC), scalar2=None,
                                            op0=ALU.mult)
                    nc.gpsimd.tensor_tensor(out=cnt[:], in0=cnt[:], in1=tmp[:],
                                            op=ALU.subtract)
                pad_count_fix(cgt, t0hi, ALU.is_gt)
                pad_count_fix(clt, t0lo, ALU.is_lt)

                # Newton step: t1 = t0 +/- (c - k) * sig / A
                def newton(tout, t0_, cnt, sgn_):
                    d = st("tmpb")
                    nc.gpsimd.tensor_scalar(out=d[:], in0=cnt[:],
                                            scalar1=col(C_K), scalar2=None,
                                            op0=ALU.subtract)
                    nc.gpsimd.tensor_tensor(out=d[:], in0=d[:], in1=sig[:],
                                            op=ALU.mult)
                    nc.gpsimd.tensor_scalar(out=d[:], in0=d[:],
                                            scalar1=col(C_INVA), scalar2=None,
                                            op0=ALU.mult)
                    nc.gpsimd.tensor_tensor(out=tout[:], in0=t0_[:], in1=d[:],
                                            op=(ALU.add if sgn_ > 0
                                                else ALU.subtract))
                t1hi = st("t1hi")
                newton(t1hi, t0hi, cgt, +1)
                t1lo = st("t1lo")
                newton(t1lo, t0lo, clt, -1)

                # S_hi on ACT: sum relu(x - t1hi); S_lo on DVE via sum min(x, t1lo)
                negthi = st("negthi")
                nc.gpsimd.tensor_scalar(out=negthi[:], in0=t1hi[:],
                                        scalar1=-1.0, scalar2=None,
                                        op0=ALU.mult)
                shi = st("shi")
                smin = st("smin")
                for ft in range(HT):
                    scr = scrtile("a")
                    nc.scalar.activation(scr[:, 0:fdi], esb[ft][:], AF.Relu,
                                         bias=negthi[:, ft:ft + 1], scale=1.0,
                                         accum_out=shi[:, ft:ft + 1])
                    scr2 = scrtile("d")
                    nc.vector.tensor_scalar(
                        out=scr2[:, 0:fdi], in0=esb[ft][:],
                        scalar1=t1lo[:, ft:ft + 1], scalar2=0.0,
                        op0=ALU.min, op1=ALU.add,
                        accum_out=smin[:, ft:ft + 1])

                # pad fixes:
                #   shi -= padc * relu(pdev - t1hi)
                #   smin -= padc * min(pdev, t1lo)
                tmp = st("tmpa")
                nc.gpsimd.tensor_tensor(out=tmp[:], in0=pdev[:], in1=t1hi[:],
                                        op=ALU.subtract)
                nc.gpsimd.tensor_scalar(out=tmp[:], in0=tmp[:], scalar1=0.0,
                                        scalar2=None, op0=ALU.max)
                nc.gpsimd.tensor_scalar(out=tmp[:], in0=tmp[:],
                                        scalar1=col(C_PADC), scalar2=None,
                                        op0=ALU.mult)
                nc.gpsimd.tensor_tensor(out=shi[:], in0=shi[:], in1=tmp[:],
                                        op=ALU.subtract)
                tmp2 = st("tmpa")
                nc.vector.tensor_tensor(out=tmp2[:], in0=pdev[:], in1=t1lo[:],
                                        op=ALU.min)
                nc.gpsimd.tensor_scalar(out=tmp2[:], in0=tmp2[:],
                                        scalar1=col(C_PADC), scalar2=None,
                                        op0=ALU.mult)
                nc.gpsimd.tensor_tensor(out=smin[:], in0=smin[:], in1=tmp2[:],
                                        op=ALU.subtract)
                # S_lo = L * t1lo - smin_valid
                slo = st("slo")
                nc.gpsimd.tensor_scalar(out=slo[:], in0=t1lo[:],
                                        scalar1=col(C_L), scalar2=None,
                                        op0=ALU.mult)
                nc.gpsimd.tensor_tensor(out=slo[:], in0=slo[:], in1=smin[:],
                                        op=ALU.subtract)

                # assemble: trimmed = (sumx - top - bot) / denom
                top = st("top")
                nc.gpsimd.tensor_scalar(out=top[:], in0=t1hi[:],
                                        scalar1=col(C_K), scalar2=None,
                                        op0=ALU.mult)
                nc.gpsimd.tensor_tensor(out=top[:], in0=top[:], in1=shi[:],
                                        op=ALU.add)
                bot = st("bot")
                nc.gpsimd.tensor_scalar(out=bot[:], in0=t1lo[:],
                                        scalar1=col(C_K), scalar2=None,
                                        op0=ALU.mult)
                nc.gpsimd.tensor_tensor(out=bot[:], in0=bot[:], in1=slo[:],
                                        op=ALU.subtract)
                trm = st("trm")
                nc.gpsimd.tensor_tensor(out=trm[:], in0=sumx[:], in1=top[:],
                                        op=ALU.subtract)
                nc.gpsimd.tensor_tensor(out=trm[:], in0=trm[:], in1=bot[:],
                                        op=ALU.subtract)
                nc.vector.tensor_scalar(
                    out=trimmed[:, s * HT:(s + 1) * HT], in0=trm[:],
                    scalar1=col(C_INVDEN), scalar2=None, op0=ALU.mult)

            # ---- software-pipelined emission --------------------------------
            pend = {}
            for s in range(SPC + 1):
                if s < SPC:
                    pend[s] = emit_encode(s)
                if s >= 1:
                    emit_select(s - 1, *pend.pop(s - 1))

            # ---- decode (batched over samples) ------------------------------
            trT = trimmed.rearrange("p (s f) -> p f s", f=HT)
            for ht3 in range(HT):
                w3c = pws.tile([128, HT * 128], F32R, tag="w3c",
                               name=f"w3c{ht3}")
                nc.sync.dma_start(
                    out=w3c.rearrange("p (t q) -> p t q", t=HT),
                    in_=W3[:, ht3 * 128:(ht3 + 1) * 128]
                        .rearrange("(t p) h -> p t h", p=128))
                dp = ps_tp.tile([128, SPC], F32, tag="tp", name=f"dp{ht3}")
                for kt in range(HT):
                    nc.tensor.matmul(
                        dp[:], w3c[:, kt * 128:(kt + 1) * 128], trT[:, kt, :],
                        start=(kt == 0), stop=(kt == HT - 1))
                nc.vector.tensor_scalar(
                    out=h3sb[:, ht3 * SPC:(ht3 + 1) * SPC], in0=dp[:],
                    scalar1=b3t[:, ht3:ht3 + 1], scalar2=0.0,
                    op0=ALU.add, op1=ALU.max)
            op_ = ps_h.tile([NOUT, SPC], F32, tag="hp", name="op_")
            for ht3 in range(HT):
                nc.tensor.matmul(
                    op_[:], w4[:, ht3 * NOUT:(ht3 + 1) * NOUT],
                    h3sb[:, ht3 * SPC:(ht3 + 1) * SPC],
                    start=(ht3 == 0), stop=(ht3 == HT - 1))
            outsb = pc.tile([NOUT, SPC], F32, tag="outsb")
            nc.scalar.activation(outsb[:], op_[:], AF.Identity, bias=b4t[:],
                                 scale=1.0)
            nc.sync.dma_start(out=Y[:], in_=outsb[:])

    nc.compile()
    _BUILD_CACHE[fds] = nc
    return nc


def kernel(**inputs):
    X = np.ascontiguousarray(np.asarray(inputs["X"], dtype=np.float32))
    mask = np.asarray(inputs["mask"], dtype=np.float32)
    W1 = np.ascontiguousarray(np.asarray(inputs["W1"], dtype=np.float32))
    b1 = np.asarray(inputs["b1"], dtype=np.float32)
    W2 = np.ascontiguousarray(np.asarray(inputs["W2"], dtype=np.float32))
    b2 = np.asarray(inputs["b2"], dtype=np.float32)
    W3 = np.ascontiguousarray(np.asarray(inputs["W3"], dtype=np.float32))
    b3 = np.asarray(inputs["b3"], dtype=np.float32)
    W4 = np.ascontiguousarray(np.asarray(inputs["W4"], dtype=np.float32))
    b4 = np.asarray(inputs["b4"], dtype=np.float32).reshape(-1)

    L = mask.sum(axis=1).astype(np.int64)                  # [B]
    k = np.floor(L.astype(np.float64) * TRIM_RATIO).astype(np.int64)
    Xm = X * mask[:, :, None]                              # zero pad rows

    # sorted round-robin slot assignment: slot s of core c gets sample
    # order[s * NCORES + c]; slot FD = max L in slot rounded up to 128.
    order = np.argsort(-L, kind="stable")
    fds = []
    for s in range(SPC):
        grp = order[s * NCORES:(s + 1) * NCORES]
        fds.append(int(min(N, -(-int(L[grp].max()) // 128) * 128)))
    fds = tuple(fds)

    CONST = np.zeros((NCORES, SPC, 128, NCC), np.float32)
    Xc = np.zeros((NCORES, SPC, N, D_IN), np.float32)
    for s in range(SPC):
        for c in range(NCORES):
            bidx = int(order[s * NCORES + c])
            Lb, kb = float(L[bidx]), float(k[bidx])
            CONST[c, s, :, C_INVL] = 1.0 / Lb
            z = _norm_ppf(1.0 - kb / Lb) if kb > 0 else 3.0
            CONST[c, s, :, C_Z] = z
            CONST[c, s, :, C_K] = kb
            phi = np.exp(-0.5 * z * z) / np.sqrt(2 * np.pi)
            CONST[c, s, :, C_INVA] = 1.0 / (Lb * phi)
            CONST[c, s, :, C_INVDEN] = 1.0 / (Lb - 2.0 * kb)
            CONST[c, s, :, C_PADC] = float(fds[s] - L[bidx])
            CONST[c, s, :, C_L] = Lb
            Xc[c, s] = Xm[bidx]

    nc = _build_program(fds)
    shared = {
        "W1": W1, "W2": W2, "W3": W3, "W4": W4,
        "B1": np.ascontiguousarray(b1.reshape(HT, 128).T),
        "B2": np.ascontiguousarray(b2.reshape(HT, 128).T),
        "B3": np.ascontiguousarray(b3.reshape(HT, 128).T),
        "B4": np.ascontiguousarray(b4.reshape(NOUT, 1)),
    }
    in_maps = []
    for c in range(NCORES):
        m = dict(shared)
        m["X"] = np.ascontiguousarray(Xc[c])
        m["CONST"] = np.ascontiguousarray(CONST[c])
        in_maps.append(m)

    res = run_bass_kernel_spmd(nc, in_maps, list(range(NCORES)), trace=_TRACE)
    _BUILD_CACHE["last_res"] = res
    out = np.zeros((B, NOUT), np.float32)
    for s in range(SPC):
        for c in range(NCORES):
            out[int(order[s * NCORES + c]), :] = res.results[c]["Y"][:, s]
    return out


# revision 13
# speedup vs baseline: 1.0217x; 1.0217x over previous
"""Trainium2 Bass kernel for nn_DeepSet_TM (DeepSet encode MLP -> per-feature
trimmed mean over ragged N -> decode MLP).

Strategy:
  - Data-parallel over B: 8 samples per core on 8 cores, identical SPMD
    program. Samples are sorted by valid length L and dealt round-robin to
    cores, so slot s on every core has a similar L; the slot's free-dim FD
    (max L in slot, rounded up to 128) is baked into the program, which cuts
    all matmul/scan work by ~25% on average. Per-sample scalars arrive as
    input tensors, so one compiled program serves any mask with the same
    slot-FD signature.
  - Matmuls in float32r (TF32-grade, 1 cyc/row) with transposed layouts so the
    per-feature reduction axis (n) lands on the free dimension.
  - Trimmed mean without sorting, via the CVaR duality
        sum_of_k_largest = min_t [ sum relu(x - t) + k t ],
    which is flat (2nd order) around the optimal t. t is initialized from
    per-feature mean/std (Gaussian quantile) and refined with one Newton step
    on exact counts: count_gt on VectorE (is_gt + accumulate), count_lt on
    ScalarE (Sign + accumulate). Final tails: S_hi on ScalarE (Relu +
    accumulate), S_lo on VectorE via S_lo(t) = L*t - sum_valid min(x, t).
  - Ragged handling: X pad rows are zeroed on host, so padded e-columns equal
    a per-feature constant; its exact (f32r) value is read back from the last
    e-column on device and used for exact pad corrections of all stats.
  - Software pipelining: encode(s) is emitted before select(s-1) so the PE and
    the evacuation engines stay ahead of the selection passes.
"""
import numpy as np

import concourse.bacc as bacc
import concourse.mybir as mybir
from concourse import masks
from concourse.tile import TileContext
from concourse.bass_utils import run_bass_kernel_spmd

B, N, D_IN, D_H, NOUT = 64, 1024, 512, 1024, 10
TRIM_RATIO = 0.1
NCORES = 8
SPC = B // NCORES          # samples (slots) per core
CH = 512                   # max n-chunk (f32 matmul moving-operand limit)
DT = D_IN // 128           # 4  d-tiles
HT = D_H // 128            # 8  h/f-tiles
SUBN = 256                 # always-valid prefix used for the variance estimate
F32 = mybir.dt.float32
F32R = mybir.dt.float32r
AF = mybir.ActivationFunctionType
ALU = mybir.AluOpType

# CONST columns (per-sample scalars, replicated over partitions)
C_INVL, C_Z, C_K, C_INVA, C_INVDEN, C_PADC, C_L = 0, 1, 2, 3, 4, 5, 6
NCC = 7


def _norm_ppf(p):
    """Acklam's rational approximation of the standard normal inverse CDF."""
    a = [-3.969683028665376e+01, 2.209460984245205e+02, -2.759285104469687e+02,
         1.383577518672690e+02, -3.066479806614716e+01, 2.506628277459239e+00]
    b = [-5.447609879822406e+01, 1.615858368580409e+02, -1.556989798598866e+02,
         6.680131188771972e+01, -1.328068155288572e+01]
    c = [-7.784894002430293e-03, -3.223964580411365e-01, -2.400758277161838e+00,
         -2.549732539343734e+00, 4.374664141464968e+00, 2.938163982698783e+00]
    d = [7.784695709041462e-03, 3.224671290700398e-01, 2.445134137142996e+00,
         3.754408661907416e+00]
    p = float(p)
    if p < 0.02425:
        q = np.sqrt(-2 * np.log(p))
        return (((((c[0]*q+c[1])*q+c[2])*q+c[3])*q+c[4])*q+c[5]) / \
               ((((d[0]*q+d[1])*q+d[2])*q+d[3])*q+1)
    if p > 1 - 0.02425:
        return -_norm_ppf(1 - p)
    q = p - 0.5
    r = q * q
    return (((((a[0]*r+a[1])*r+a[2])*r+a[3])*r+a[4])*r+a[5])*q / \
           (((((b[0]*r+b[1])*r+b[2])*r+b[3])*r+b[4])*r+1)


def _chunks(fd):
    out = [CH] * (fd // CH)
    if fd % CH:
        out.append(fd % CH)
    return out


_BUILD_CACHE = {}
_TRACE = False


def _build_program(fds):
    if fds in _BUILD_CACHE:
        return _BUILD_CACHE[fds]
    nc = bacc.Bacc("TRN2", target_bir_lowering=False, debug=False)

    X = nc.declare_dram_parameter("X", [SPC, N, D_IN], F32R, isOutput=False)
    W1 = nc.declare_dram_parameter("W1", [D_IN, D_H], F32R, isOutput=False)
    W2 = nc.declare_dram_parameter("W2", [D_H, D_H], F32R, isOutput=False)
    W3 = nc.declare_dram_parameter("W3", [D_H, D_H], F32R, isOutput=False)
    W4 = nc.declare_dram_parameter("W4", [D_H, NOUT], F32R, isOutput=False)
    B1 = nc.declare_dram_parameter("B1", [128, HT], F32, isOutput=False)
    B2 = nc.declare_dram_parameter("B2", [128, HT], F32, isOutput=False)
    B3 = nc.declare_dram_parameter("B3", [128, HT], F32, isOutput=False)
    B4 = nc.declare_dram_parameter("B4", [NOUT, 1], F32, isOutput=False)
    CONST = nc.declare_dram_parameter("CONST", [SPC, 128, NCC], F32,
                                      isOutput=False)
    Y = nc.declare_dram_parameter("Y", [NOUT, SPC], F32, isOutput=True)

    with TileContext(nc) as tc:
        with (
            tc.tile_pool(name="const", bufs=1) as pc,
            tc.tile_pool(name="xraw", bufs=2) as pxr,
            tc.tile_pool(name="xt", bufs=1) as pxt,
            tc.tile_pool(name="h1", bufs=2) as ph1,
            tc.tile_pool(name="epool", bufs=2) as pe,
            tc.tile_pool(name="scr", bufs=2) as pscr,
            tc.tile_pool(name="stats", bufs=2) as pst,
            tc.tile_pool(name="wstr", bufs=2) as pws,
            tc.tile_pool(name="ps_tp", bufs=2, space="PSUM") as ps_tp,
            tc.tile_pool(name="ps_h", bufs=2, space="PSUM") as ps_h,
            tc.tile_pool(name="ps_e", bufs=2, space="PSUM") as ps_e,
        ):
            # ---- first X chunk before the weights: un-gates the transposes --
            xr0 = pxr.tile([128, DT * CH], F32R, tag="xr")
            nc.sync.dma_start(
                out=xr0.rearrange("p (t d) -> p t d", t=DT),
                in_=X[0, 0:CH, :].rearrange("(t p) d -> p t d", p=128))

            # ---- resident constants / weights -------------------------------
            w1 = pc.tile([128, DT * D_H], F32R, tag="w1")
            nc.sync.dma_start(out=w1.rearrange("p (t h) -> p t h", t=DT),
                              in_=W1.rearrange("(t p) h -> p t h", p=128))
            w2 = pc.tile([128, HT * D_H], F32R, tag="w2")
            nc.sync.dma_start(out=w2.rearrange("p (t h) -> p t h", t=HT),
                              in_=W2.rearrange("(t p) h -> p t h", p=128))
            w4 = pc.tile([128, HT * NOUT], F32R, tag="w4")
            nc.sync.dma_start(out=w4.rearrange("p (t o) -> p t o", t=HT),
                              in_=W4.rearrange("(t p) o -> p t o", p=128))
            b1t = pc.tile([128, HT], F32, tag="b1t")
            nc.sync.dma_start(out=b1t[:], in_=B1[:])
            b2t = pc.tile([128, HT], F32, tag="b2t")
            nc.sync.dma_start(out=b2t[:], in_=B2[:])
            b3t = pc.tile([128, HT], F32, tag="b3t")
            nc.sync.dma_start(out=b3t[:], in_=B3[:])
            b4t = pc.tile([NOUT, 1], F32, tag="b4t")
            nc.sync.dma_start(out=b4t[:], in_=B4[:])
            ident = pc.tile([128, 128], F32, tag="ident")
            masks.make_identity(nc, ident[:])
            trimmed = pc.tile([128, SPC * HT], F32R, tag="trimmed")
            h3sb = pc.tile([128, HT * SPC], F32R, tag="h3sb")

            _stn = [0]

            def st(tag, cols=HT):
                _stn[0] += 1
                return pst.tile([128, cols], F32, tag=tag,
                                name=f"st_{tag}_{_stn[0]}")

            def scrtile(eng):
                _stn[0] += 1
                return pscr.tile([128, N], F32, tag=f"scr_{eng}",
                                 name=f"scr_{eng}_{_stn[0]}")

            # ================= encode stage ==================================
            def emit_encode(s):
                fd = fds[s]
                chs = _chunks(fd)
                cst = pst.tile([128, NCC], F32, tag="cst", name=f"cst{s}")
                nc.sync.dma_start(out=cst[:], in_=CONST[s])
                esb = [pe.tile([128, fd], F32, tag=f"e{ft}",
                               name=f"esb{s}_{ft}") for ft in range(HT)]
                sumxc = st("sumxc", HT * 2)
                n0 = 0
                for c, cfd in enumerate(chs):
                    ntt = cfd // 128
                    if s == 0 and c == 0:
                        xr = xr0
                    else:
                        xr = pxr.tile([128, DT * CH], F32R, tag="xr",
                                      name=f"xr{s}_{c}")
                        nc.sync.dma_start(
                            out=xr.rearrange("p (t d) -> p t d", t=DT)
                                [:, 0:ntt, :],
                            in_=X[s, n0:n0 + cfd, :]
                                .rearrange("(t p) d -> p t d", p=128))
                    xt = pxt.tile([128, DT * CH], F32R, tag="xt",
                                  name=f"xt{s}_{c}")
                    for dt in range(DT):
                        tp = ps_tp.tile([128, CH], F32, tag="tp",
                                        name=f"tp{s}_{c}_{dt}")
                        for nt in range(ntt):
                            nc.tensor.transpose(
                                tp[:, nt * 128:(nt + 1) * 128],
                                xr[:, nt * D_IN + dt * 128:
                                      nt * D_IN + (dt + 1) * 128].bitcast(F32),
                                ident[:])
                        nc.vector.tensor_copy(
                            xt[:, dt * CH:dt * CH + cfd], tp[:, 0:cfd])
                    h1 = ph1.tile([128, HT * CH], F32R, tag="h1",
                                  name=f"h1_{s}_{c}")
                    for ht in range(HT):
                        hp = ps_h.tile([128, CH], F32, tag="hp",
                                       name=f"hp{s}_{c}_{ht}")
                        for dt in range(DT):
                            nc.tensor.matmul(
                                hp[:, 0:cfd],
                                w1[:, dt * D_H + ht * 128:
                                      dt * D_H + (ht + 1) * 128],
                                xt[:, dt * CH:dt * CH + cfd],
                                start=(dt == 0), stop=(dt == DT - 1))
                        nc.vector.tensor_scalar(
                            out=h1[:, ht * CH:ht * CH + cfd], in0=hp[:, 0:cfd],
                            scalar1=b1t[:, ht:ht + 1], scalar2=0.0,
                            op0=ALU.add, op1=ALU.max)
                    for ft in range(HT):
                        ep = ps_e.tile([128, CH], F32, tag="ep",
                                       name=f"ep{s}_{c}_{ft}")
                        for ht in range(HT):
                            nc.tensor.matmul(
                                ep[:, 0:cfd],
                                w2[:, ht * D_H + ft * 128:
                                      ht * D_H + (ft + 1) * 128],
                                h1[:, ht * CH:ht * CH + cfd],
                                start=(ht == 0), stop=(ht == HT - 1))
                        nc.scalar.activation(
                            esb[ft][:, n0:n0 + cfd], ep[:, 0:cfd], AF.Identity,
                            bias=b2t[:, ft:ft + 1], scale=1.0,
                            accum_out=sumxc[:, ft * 2 + c:ft * 2 + c + 1])
                    n0 += cfd
                return cst, esb, sumxc, len(chs)

            # ================= selection stage ===============================
            def emit_select(s, cst, esb, sumxc, nch):
                fd = float(fds[s])
                fdi = fds[s]
                col = lambda j: cst[:, j:j + 1]
                # exact pad value: last e column (a pad whenever padc > 0)
                pdev = st("pdev")
                for ft in range(HT):
                    nc.gpsimd.tensor_copy(pdev[:, ft:ft + 1],
                                          esb[ft][:, fdi - 1:fdi])
                padp = st("padp")
                nc.gpsimd.tensor_scalar(out=padp[:], in0=pdev[:],
                                        scalar1=col(C_PADC), scalar2=None,
                                        op0=ALU.mult)
                sumx = st("sumx")
                if nch == 2:
                    nc.gpsimd.tensor_tensor(
                        out=sumx[:],
                        in0=sumxc.rearrange("p (f c) -> p c f", c=2)[:, 0, :],
                        in1=sumxc.rearrange("p (f c) -> p c f", c=2)[:, 1, :],
                        op=ALU.add)
                else:
                    nc.gpsimd.tensor_copy(
                        sumx[:],
                        sumxc.rearrange("p (f c) -> p c f", c=2)[:, 0, :])
                nc.gpsimd.tensor_tensor(out=sumx[:], in0=sumx[:], in1=padp[:],
                                        op=ALU.subtract)
                # variance from the always-valid first SUBN columns
                sumxx = st("sumxx")
                for ft in range(HT):
                    scr = scrtile("a")
                    nc.scalar.activation(scr[:, 0:SUBN], esb[ft][:, 0:SUBN],
                                         AF.Square,
                                         accum_out=sumxx[:, ft:ft + 1])
                mu = st("mu")
                nc.gpsimd.tensor_scalar(out=mu[:], in0=sumx[:],
                                        scalar1=col(C_INVL), scalar2=None,
                                        op0=ALU.mult)
                var = st("var")
                nc.gpsimd.tensor_scalar(out=var[:], in0=sumxx[:],
                                        scalar1=1.0 / SUBN, scalar2=None,
                                        op0=ALU.mult)
                musq = st("musq")
                nc.gpsimd.tensor_tensor(out=musq[:], in0=mu[:], in1=mu[:],
                                        op=ALU.mult)
                nc.gpsimd.tensor_tensor(out=var[:], in0=var[:], in1=musq[:],
                                        op=ALU.subtract)
                nc.gpsimd.tensor_scalar(out=var[:], in0=var[:], scalar1=1e-12,
                                        scalar2=None, op0=ALU.max)
                sig = st("sig")
                nc.scalar.activation(sig[:], var[:], AF.Sqrt)
                sigz = st("sigz")
                nc.gpsimd.tensor_scalar(out=sigz[:], in0=sig[:],
                                        scalar1=col(C_Z), scalar2=None,
                                        op0=ALU.mult)
                t0hi = st("t0hi")
                nc.gpsimd.tensor_tensor(out=t0hi[:], in0=mu[:], in1=sigz[:],
                                        op=ALU.add)
                t0lo = st("t0lo")
                nc.gpsimd.tensor_tensor(out=t0lo[:], in0=mu[:], in1=sigz[:],
                                        op=ALU.subtract)
                negt0lo = st("negt0lo")
                nc.gpsimd.tensor_scalar(out=negt0lo[:], in0=t0lo[:],
                                        scalar1=-1.0, scalar2=None,
                                        op0=ALU.mult)

                # counts at t0: c_gt on DVE (is_gt), c_lt on ACT (Sign)
                cgt = st("cgt")
                sgn = st("sgn")
                for ft in range(HT):
                    scr = scrtile("d")
                    nc.vector.tensor_scalar(
                        out=scr[:, 0:fdi], in0=esb[ft][:],
                        scalar1=t0hi[:, ft:ft + 1],
                        scalar2=0.0, op0=ALU.is_gt, op1=ALU.add,
                        accum_out=cgt[:, ft:ft + 1])
                    scr2 = scrtile("a")
                    nc.scalar.activation(
                        scr2[:, 0:fdi], esb[ft][:], AF.Sign,
                        bias=negt0lo[:, ft:ft + 1], scale=1.0,
                        accum_out=sgn[:, ft:ft + 1])
                # c_lt_all = (FD - sgn_sum)/2
                clt = st("clt")
                nc.gpsimd.tensor_scalar(out=clt[:], in0=sgn[:],
                                        scalar1=-0.5, scalar2=fd / 2.0,
                                        op0=ALU.mult, op1=ALU.add)

                def pad_count_fix(cnt, thr, op):
                    tmp = st("tmpa")
                    nc.vector.tensor_tensor(out=tmp[:], in0=pdev[:], in1=thr[:],
                                            op=op)
                    nc.gpsimd.tensor_scalar(out=tmp[:], in0=tmp[:],
                                            scalar1=col(C_PADC), scalar2=None,
                                            op0=ALU.mult)
                    nc.gpsimd.tensor_tensor(out=cnt[:], in0=cnt[:], in1=tmp[:],
                                            op=ALU.subtract)
                pad_count_fix(cgt, t0hi, ALU.is_gt)
                pad_count_fix(clt, t0lo, ALU.is_lt)

                # Newton step: t1 = t0 +/- (c - k) * sig / A
                def newton(tout, t0_, cnt, sgn_):
                    d = st("tmpb")
                    nc.gpsimd.tensor_scalar(out=d[:], in0=cnt[:],
                                            scalar1=col(C_K), scalar2=None,
                                            op0=ALU.subtract)
                    nc.gpsimd.tensor_tensor(out=d[:], in0=d[:], in1=sig[:],
                                            op=ALU.mult)
                    nc.gpsimd.tensor_scalar(out=d[:], in0=d[:],
                                            scalar1=col(C_INVA), scalar2=None,
                                            op0=ALU.mult)
                    nc.gpsimd.tensor_tensor(out=tout[:], in0=t0_[:], in1=d[:],
                                            op=(ALU.add if sgn_ > 0
                                                else ALU.subtract))
                t1hi = st("t1hi")
                newton(t1hi, t0hi, cgt, +1)
                t1lo = st("t1lo")
                newton(t1lo, t0lo, clt, -1)

                # S_hi on ACT: sum relu(x - t1hi); S_lo on DVE via sum min(x, t1lo)
                negthi = st("negthi")
                nc.gpsimd.tensor_scalar(out=negthi[:], in0=t1hi[:],
                                        scalar1=-1.0, scalar2=None,
                                        op0=ALU.mult)
                shi = st("shi")
                smin = st("smin")
                for ft in range(HT):
                    scr = scrtile("a")
                    nc.scalar.activation(scr[:, 0:fdi], esb[ft][:], AF.Relu,
                                         bias=negthi[:, ft:ft + 1], scale=1.0,
                                         accum_out=shi[:, ft:ft + 1])
                    scr2 = scrtile("d")
                    nc.vector.tensor_scalar(
                        out=scr2[:, 0:fdi], in0=esb[ft][:],
                        scalar1=t1lo[:, ft:ft + 1], scalar2=0.0,
                        op0=ALU.min, op1=ALU.add,
                        accum_out=smin[:, ft:ft + 1])

                # pad fixes:
                #   shi -= padc * relu(pdev - t1hi)
                #   smin -= padc * min(pdev, t1lo)
                tmp = st("tmpa")
                nc.gpsimd.tensor_tensor(out=tmp[:], in0=pdev[:], in1=t1hi[:],
                                        op=ALU.subtract)
                nc.gpsimd.tensor_scalar(out=tmp[:], in0=tmp[:], scalar1=0.0,
                                        scalar2=None, op0=ALU.max)
                nc.gpsimd.tensor_scalar(out=tmp[:], in0=tmp[:],
                                        scalar1=col(C_PADC), scalar2=None,
                                        op0=ALU.mult)
                nc.gpsimd.tensor_tensor(out=shi[:], in0=shi[:], in1=tmp[:],
                                        op=ALU.subtract)
                tmp2 = st("tmpa")
                nc.vector.tensor_tensor(out=tmp2[:], in0=pdev[:], in1=t1lo[:],
                                        op=ALU.min)
                nc.gpsimd.tensor_scalar(out=tmp2[:], in0=tmp2[:],
                                        scalar1=col(C_PADC), scalar2=None,
                                        op0=ALU.mult)
                nc.gpsimd.tensor_tensor(out=smin[:], in0=smin[:], in1=tmp2[:],
                                        op=ALU.subtract)
                # S_lo = L * t1lo - smin_valid
                slo = st("slo")
                nc.gpsimd.tensor_scalar(out=slo[:], in0=t1lo[:],
                                        scalar1=col(C_L), scalar2=None,
                                        op0=ALU.mult)
                nc.gpsimd.tensor_tensor(out=slo[:], in0=slo[:], in1=smin[:],
                                        op=ALU.subtract)

                # assemble: trimmed = (sumx - top - bot) / denom
                top = st("top")
                nc.gpsimd.tensor_scalar(out=top[:], in0=t1hi[:],
                                        scalar1=col(C_K), scalar2=None,
                                        op0=ALU.mult)
                nc.gpsimd.tensor_tensor(out=top[:], in0=top[:], in1=shi[:],
                                        op=ALU.add)
                bot = st("bot")
                nc.gpsimd.tensor_scalar(out=bot[:], in0=t1lo[:],
                                        scalar1=col(C_K), scalar2=None,
                                        op0=ALU.mult)
                nc.gpsimd.tensor_tensor(out=bot[:], in0=bot[:], in1=slo[:],
                                        op=ALU.subtract)
                trm = st("trm")
                nc.gpsimd.tensor_tensor(out=trm[:], in0=sumx[:], in1=top[:],
                                        op=ALU.subtract)
                nc.gpsimd.tensor_tensor(out=trm[:], in0=trm[:], in1=bot[:],
                                        op=ALU.subtract)
                nc.vector.tensor_scalar(
                    out=trimmed[:, s * HT:(s + 1) * HT], in0=trm[:],
                    scalar1=col(C_INVDEN), scalar2=None, op0=ALU.mult)

            # ---- software-pipelined emission --------------------------------
            pend = {}
            for s in range(SPC + 1):
                if s < SPC:
                    pend[s] = emit_encode(s)
                if s >= 1:
                    emit_select(s - 1, *pend.pop(s - 1))

            # ---- decode (batched over samples) ------------------------------
            trT = trimmed.rearrange("p (s f) -> p f s", f=HT)
            for ht3 in range(HT):
                w3c = pws.tile([128, HT * 128], F32R, tag="w3c",
                               name=f"w3c{ht3}")
                nc.sync.dma_start(
                    out=w3c.rearrange("p (t q) -> p t q", t=HT),
                    in_=W3[:, ht3 * 128:(ht3 + 1) * 128]
                        .rearrange("(t p) h -> p t h", p=128))
                dp = ps_tp.tile([128, SPC], F32, tag="tp", name=f"dp{ht3}")
                for kt in range(HT):
                    nc.tensor.matmul(
                        dp[:], w3c[:, kt * 128:(kt + 1) * 128], trT[:, kt, :],
                        start=(kt == 0), stop=(kt == HT - 1))
                nc.vector.tensor_scalar(
                    out=h3sb[:, ht3 * SPC:(ht3 + 1) * SPC], in0=dp[:],
                    scalar1=b3t[:, ht3:ht3 + 1], scalar2=0.0,
                    op0=ALU.add, op1=ALU.max)
            op_ = ps_h.tile([NOUT, SPC], F32, tag="hp", name="op_")
            for ht3 in range(HT):
                nc.tensor.matmul(
                    op_[:], w4[:, ht3 * NOUT:(ht3 + 1) * NOUT],
                    h3sb[:, ht3 * SPC:(ht3 + 1) * SPC],
                    start=(ht3 == 0), stop=(ht3 == HT - 1))
            outsb = pc.tile([NOUT, SPC], F32, tag="outsb")
            nc.scalar.activation(outsb[:], op_[:], AF.Identity, bias=b4t[:],
                                 scale=1.0)
            nc.sync.dma_start(out=Y[:], in_=outsb[:])

    nc.compile()
    _BUILD_CACHE[fds] = nc
    return nc


def kernel(**inputs):
    X = np.ascontiguousarray(np.asarray(inputs["X"], dtype=np.float32))
    mask = np.asarray(inputs["mask"], dtype=np.float32)
    W1 = np.ascontiguousarray(np.asarray(inputs["W1"], dtype=np.float32))
    b1 = np.asarray(inputs["b1"], dtype=np.float32)
    W2 = np.ascontiguousarray(np.asarray(inputs["W2"], dtype=np.float32))
    b2 = np.asarray(inputs["b2"], dtype=np.float32)
    W3 = np.ascontiguousarray(np.asarray(inputs["W3"], dtype=np.float32))
    b3 = np.asarray(inputs["b3"], dtype=np.float32)
    W4 = np.ascontiguousarray(np.asarray(inputs["W4"], dtype=np.float32))
    b4 = np.asarray(inputs["b4"], dtype=np.float32).reshape(-1)

    L = mask.sum(axis=1).astype(np.int64)                  # [B]
    k = np.floor(L.astype(np.float64) * TRIM_RATIO).astype(np.int64)
    Xm = X * mask[:, :, None]                              # zero pad rows

    # sorted round-robin slot assignment: slot s of core c gets sample
    # order[s * NCORES + c]; slot FD = max L in slot rounded up to 128.
    order = np.argsort(-L, kind="stable")
    fds = []
    for s in range(SPC):
        grp = order[s * NCORES:(s + 1) * NCORES]
        fds.append(int(min(N, -(-int(L[grp].max()) // 128) * 128)))
    fds = tuple(fds)

    CONST = np.zeros((NCORES, SPC, 128, NCC), np.float32)
    Xc = np.zeros((NCORES, SPC, N, D_IN), np.float32)
    for s in range(SPC):
        for c in range(NCORES):
            bidx = int(order[s * NCORES + c])
            Lb, kb = float(L[bidx]), float(k[bidx])
            CONST[c, s, :, C_INVL] = 1.0 / Lb
            z = _norm_ppf(1.0 - kb / Lb) if kb > 0 else 3.0
            CONST[c, s, :, C_Z] = z
            CONST[c, s, :, C_K] = kb
            phi = np.exp(-0.5 * z * z) / np.sqrt(2 * np.pi)
            CONST[c, s, :, C_INVA] = 1.0 / (Lb * phi)
            CONST[c, s, :, C_INVDEN] = 1.0 / (Lb - 2.0 * kb)
            CONST[c, s, :, C_PADC] = float(fds[s] - L[bidx])
            CONST[c, s, :, C_L] = Lb
            Xc[c, s] = Xm[bidx]

    nc = _build_program(fds)
    shared = {
        "W1": W1, "W2": W2, "W3": W3, "W4": W4,
        "B1": np.ascontiguousarray(b1.reshape(HT, 128).T),
        "B2": np.ascontiguousarray(b2.reshape(HT, 128).T),
        "B3": np.ascontiguousarray(b3.reshape(HT, 128).T),
        "B4": np.ascontiguousarray(b4.reshape(NOUT, 1)),
    }
    in_maps = []
    for c in range(NCORES):
        m = dict(shared)
        m["X"] = np.ascontiguousarray(Xc[c])
        m["CONST"] = np.ascontiguousarray(CONST[c])
        in_maps.append(m)

    res = run_bass_kernel_spmd(nc, in_maps, list(range(NCORES)), trace=_TRACE)
    _BUILD_CACHE["last_res"] = res
    out = np.zeros((B, NOUT), np.float32)
    for s in range(SPC):
        for c in range(NCORES):
            out[int(order[s * NCORES + c]), :] = res.results[c]["Y"][:, s]
    return out


# revision 14
# speedup vs baseline: 1.1027x; 1.0793x over previous
"""Trainium2 Bass kernel for nn_DeepSet_TM (DeepSet encode MLP -> per-feature
trimmed mean over ragged N -> decode MLP).

Strategy:
  - Data-parallel over B: 8 samples per core on 8 cores, identical SPMD
    program. Samples are sorted by valid length L and dealt round-robin to
    cores, so slot s on every core has a similar L; the slot's free-dim FD
    (max L in slot, rounded up to 128) is baked into the program, which cuts
    all matmul/scan work by ~25% on average. Per-sample scalars arrive as
    input tensors, so one compiled program serves any mask with the same
    slot-FD signature.
  - Matmuls in float32r (TF32-grade, 1 cyc/row) with transposed layouts so the
    per-feature reduction axis (n) lands on the free dimension.
  - Trimmed mean without sorting, via the CVaR duality
        sum_of_k_largest = min_t [ sum relu(x - t) + k t ],
    which is flat (2nd order) around the optimal t. t is initialized from
    per-feature mean/std (Gaussian quantile) and refined with one Newton step
    on exact counts: count_gt on VectorE (is_gt + accumulate), count_lt on
    ScalarE (Sign + accumulate). Final tails: S_hi on ScalarE (Relu +
    accumulate), S_lo on VectorE via S_lo(t) = L*t - sum_valid min(x, t).
  - Ragged handling: X pad rows are zeroed on host, so padded e-columns equal
    a per-feature constant; its exact (f32r) value is read back from the last
    e-column on device and used for exact pad corrections of all stats.
  - Software pipelining: encode(s) is emitted before select(s-1) so the PE and
    the evacuation engines stay ahead of the selection passes.
"""
import numpy as np

import concourse.bacc as bacc
import concourse.mybir as mybir
from concourse import masks
from concourse.tile import TileContext
from concourse.bass_utils import run_bass_kernel_spmd

B, N, D_IN, D_H, NOUT = 64, 1024, 512, 1024, 10
TRIM_RATIO = 0.1
NCORES = 8
SPC = B // NCORES          # samples (slots) per core
CH = 512                   # max n-chunk (f32 matmul moving-operand limit)
DT = D_IN // 128           # 4  d-tiles
HT = D_H // 128            # 8  h/f-tiles
SUBN = 256                 # always-valid prefix used for the variance estimate
F32 = mybir.dt.float32
F32R = mybir.dt.float32r
AF = mybir.ActivationFunctionType
ALU = mybir.AluOpType

# CONST columns (per-sample scalars, replicated over partitions)
C_INVL, C_Z, C_K, C_INVA, C_INVDEN, C_PADC, C_L = 0, 1, 2, 3, 4, 5, 6
NCC = 7


def _norm_ppf(p):
    """Acklam's rational approximation of the standard normal inverse CDF."""
    a = [-3.969683028665376e+01, 2.209460984245205e+02, -2.759285104469687e+02,
         1.383577518672690e+02, -3.066479806614716e+01, 2.506628277459239e+00]
    b = [-5.447609879822406e+01, 1.615858368580409e+02, -1.556989798598866e+02,
         6.680131188771972e+01, -1.328068155288572e+01]
    c = [-7.784894002430293e-03, -3.223964580411365e-01, -2.400758277161838e+00,
         -2.549732539343734e+00, 4.374664141464968e+00, 2.938163982698783e+00]
    d = [7.784695709041462e-03, 3.224671290700398e-01, 2.445134137142996e+00,
         3.754408661907416e+00]
    p = float(p)
    if p < 0.02425:
        q = np.sqrt(-2 * np.log(p))
        return (((((c[0]*q+c[1])*q+c[2])*q+c[3])*q+c[4])*q+c[5]) / \
               ((((d[0]*q+d[1])*q+d[2])*q+d[3])*q+1)
    if p > 1 - 0.02425:
        return -_norm_ppf(1 - p)
    q = p - 0.5
    r = q * q
    return (((((a[0]*r+a[1])*r+a[2])*r+a[3])*r+a[4])*r+a[5])*q / \
           (((((b[0]*r+b[1])*r+b[2])*r+b[3])*r+b[4])*r+1)


def _chunks(fd):
    out = [CH] * (fd // CH)
    if fd % CH:
        out.append(fd % CH)
    return out


_BUILD_CACHE = {}
_TRACE = False


def _build_program(fds):
    if fds in _BUILD_CACHE:
        return _BUILD_CACHE[fds]
    nc = bacc.Bacc("TRN2", target_bir_lowering=False, debug=False)

    X = nc.declare_dram_parameter("X", [SPC, N, D_IN], F32R, isOutput=False)
    W1 = nc.declare_dram_parameter("W1", [D_IN, D_H], F32R, isOutput=False)
    W2 = nc.declare_dram_parameter("W2", [D_H, D_H], F32R, isOutput=False)
    W3 = nc.declare_dram_parameter("W3", [D_H, D_H], F32R, isOutput=False)
    W4 = nc.declare_dram_parameter("W4", [D_H, NOUT], F32R, isOutput=False)
    B1 = nc.declare_dram_parameter("B1", [128, HT], F32, isOutput=False)
    B2 = nc.declare_dram_parameter("B2", [128, HT], F32, isOutput=False)
    B3 = nc.declare_dram_parameter("B3", [128, HT], F32, isOutput=False)
    B4 = nc.declare_dram_parameter("B4", [NOUT, 1], F32, isOutput=False)
    CONST = nc.declare_dram_parameter("CONST", [SPC, 128, NCC], F32,
                                      isOutput=False)
    Y = nc.declare_dram_parameter("Y", [NOUT, SPC], F32, isOutput=True)

    with TileContext(nc) as tc:
        with (
            tc.tile_pool(name="const", bufs=1) as pc,
            tc.tile_pool(name="xraw", bufs=2) as pxr,
            tc.tile_pool(name="xt", bufs=1) as pxt,
            tc.tile_pool(name="h1", bufs=2) as ph1,
            tc.tile_pool(name="epool", bufs=2) as pe,
            tc.tile_pool(name="scr", bufs=2) as pscr,
            tc.tile_pool(name="stats", bufs=2) as pst,
            tc.tile_pool(name="wstr", bufs=2) as pws,
            tc.tile_pool(name="ps_tp", bufs=2, space="PSUM") as ps_tp,
            tc.tile_pool(name="ps_h", bufs=2, space="PSUM") as ps_h,
            tc.tile_pool(name="ps_e", bufs=2, space="PSUM") as ps_e,
        ):
            # ---- first X chunk before the weights: un-gates the transposes --
            xr0 = pxr.tile([128, DT * CH], F32R, tag="xr")
            nc.sync.dma_start(
                out=xr0.rearrange("p (t d) -> p t d", t=DT),
                in_=X[0, 0:CH, :].rearrange("(t p) d -> p t d", p=128))

            # ---- resident constants / weights -------------------------------
            w1 = pc.tile([128, DT * D_H], F32R, tag="w1")
            nc.sync.dma_start(out=w1.rearrange("p (t h) -> p t h", t=DT),
                              in_=W1.rearrange("(t p) h -> p t h", p=128))
            w2 = pc.tile([128, HT * D_H], F32R, tag="w2")
            nc.sync.dma_start(out=w2.rearrange("p (t h) -> p t h", t=HT),
                              in_=W2.rearrange("(t p) h -> p t h", p=128))
            w4 = pc.tile([128, HT * NOUT], F32R, tag="w4")
            nc.sync.dma_start(out=w4.rearrange("p (t o) -> p t o", t=HT),
                              in_=W4.rearrange("(t p) o -> p t o", p=128))
            b1t = pc.tile([128, HT], F32, tag="b1t")
            nc.sync.dma_start(out=b1t[:], in_=B1[:])
            b2t = pc.tile([128, HT], F32, tag="b2t")
            nc.sync.dma_start(out=b2t[:], in_=B2[:])
            b3t = pc.tile([128, HT], F32, tag="b3t")
            nc.sync.dma_start(out=b3t[:], in_=B3[:])
            b4t = pc.tile([NOUT, 1], F32, tag="b4t")
            nc.sync.dma_start(out=b4t[:], in_=B4[:])
            ident = pc.tile([128, 128], F32, tag="ident")
            masks.make_identity(nc, ident[:])
            trimmed = pc.tile([128, SPC * HT], F32R, tag="trimmed")
            h3sb = pc.tile([128, HT * SPC], F32R, tag="h3sb")

            _stn = [0]

            def st(tag, cols=HT):
                _stn[0] += 1
                return pst.tile([128, cols], F32, tag=tag,
                                name=f"st_{tag}_{_stn[0]}")

            def scrtile(eng):
                _stn[0] += 1
                return pscr.tile([128, N], F32, tag=f"scr_{eng}",
                                 name=f"scr_{eng}_{_stn[0]}")

            # ================= encode stage ==================================
            def emit_encode(s):
                fd = fds[s]
                chs = _chunks(fd)
                cst = pst.tile([128, NCC], F32, tag="cst", name=f"cst{s}")
                nc.sync.dma_start(out=cst[:], in_=CONST[s])
                esb = [pe.tile([128, fd], F32, tag=f"e{ft}",
                               name=f"esb{s}_{ft}") for ft in range(HT)]
                sumxc = st("sumxc", HT * 2)
                n0 = 0
                for c, cfd in enumerate(chs):
                    ntt = cfd // 128
                    if s == 0 and c == 0:
                        xr = xr0
                    else:
                        xr = pxr.tile([128, DT * CH], F32R, tag="xr",
                                      name=f"xr{s}_{c}")
                        nc.sync.dma_start(
                            out=xr.rearrange("p (t d) -> p t d", t=DT)
                                [:, 0:ntt, :],
                            in_=X[s, n0:n0 + cfd, :]
                                .rearrange("(t p) d -> p t d", p=128))
                    xt = pxt.tile([128, DT * CH], F32R, tag="xt",
                                  name=f"xt{s}_{c}")
                    for dt in range(DT):
                        tp = ps_tp.tile([128, CH], F32, tag="tp",
                                        name=f"tp{s}_{c}_{dt}")
                        for nt in range(ntt):
                            nc.tensor.transpose(
                                tp[:, nt * 128:(nt + 1) * 128],
                                xr[:, nt * D_IN + dt * 128:
                                      nt * D_IN + (dt + 1) * 128].bitcast(F32),
                                ident[:])
                        nc.vector.tensor_copy(
                            xt[:, dt * CH:dt * CH + cfd], tp[:, 0:cfd])
                    h1 = ph1.tile([128, HT * CH], F32R, tag="h1",
                                  name=f"h1_{s}_{c}")
                    for ht in range(HT):
                        hp = ps_h.tile([128, CH], F32, tag="hp",
                                       name=f"hp{s}_{c}_{ht}")
                        for dt in range(DT):
                            nc.tensor.matmul(
                                hp[:, 0:cfd],
                                w1[:, dt * D_H + ht * 128:
                                      dt * D_H + (ht + 1) * 128],
                                xt[:, dt * CH:dt * CH + cfd],
                                start=(dt == 0), stop=(dt == DT - 1))
                        nc.vector.tensor_scalar(
                            out=h1[:, ht * CH:ht * CH + cfd], in0=hp[:, 0:cfd],
                            scalar1=b1t[:, ht:ht + 1], scalar2=0.0,
                            op0=ALU.add, op1=ALU.max)
                    for ft in range(HT):
                        ep = ps_e.tile([128, CH], F32, tag="ep",
                                       name=f"ep{s}_{c}_{ft}")
                        for ht in range(HT):
                            nc.tensor.matmul(
                                ep[:, 0:cfd],
                                w2[:, ht * D_H + ft * 128:
                                      ht * D_H + (ft + 1) * 128],
                                h1[:, ht * CH:ht * CH + cfd],
                                start=(ht == 0), stop=(ht == HT - 1))
                        nc.scalar.activation(
                            esb[ft][:, n0:n0 + cfd], ep[:, 0:cfd], AF.Identity,
                            bias=b2t[:, ft:ft + 1], scale=1.0,
                            accum_out=sumxc[:, ft * 2 + c:ft * 2 + c + 1])
                    n0 += cfd
                return cst, esb, sumxc, len(chs)

            # ================= selection stage ===============================
            def emit_select(s, cst, esb, sumxc, nch):
                fd = float(fds[s])
                fdi = fds[s]
                col = lambda j: cst[:, j:j + 1]
                # exact pad value: last e column (a pad whenever padc > 0)
                pdev = st("pdev")
                for ft in range(HT):
                    nc.vector.tensor_copy(pdev[:, ft:ft + 1],
                                          esb[ft][:, fdi - 1:fdi])
                padp = st("padp")
                nc.vector.tensor_scalar(out=padp[:], in0=pdev[:],
                                        scalar1=col(C_PADC), scalar2=None,
                                        op0=ALU.mult)
                sumx = st("sumx")
                if nch == 2:
                    nc.vector.tensor_tensor(
                        out=sumx[:],
                        in0=sumxc.rearrange("p (f c) -> p c f", c=2)[:, 0, :],
                        in1=sumxc.rearrange("p (f c) -> p c f", c=2)[:, 1, :],
                        op=ALU.add)
                else:
                    nc.vector.tensor_copy(
                        sumx[:],
                        sumxc.rearrange("p (f c) -> p c f", c=2)[:, 0, :])
                nc.vector.tensor_tensor(out=sumx[:], in0=sumx[:], in1=padp[:],
                                        op=ALU.subtract)
                # variance from the always-valid first SUBN columns
                sumxx = st("sumxx")
                for ft in range(HT):
                    scr = scrtile("a")
                    nc.scalar.activation(scr[:, 0:SUBN], esb[ft][:, 0:SUBN],
                                         AF.Square,
                                         accum_out=sumxx[:, ft:ft + 1])
                mu = st("mu")
                nc.vector.tensor_scalar(out=mu[:], in0=sumx[:],
                                        scalar1=col(C_INVL), scalar2=None,
                                        op0=ALU.mult)
                var = st("var")
                nc.vector.tensor_scalar(out=var[:], in0=sumxx[:],
                                        scalar1=1.0 / SUBN, scalar2=None,
                                        op0=ALU.mult)
                musq = st("musq")
                nc.vector.tensor_tensor(out=musq[:], in0=mu[:], in1=mu[:],
                                        op=ALU.mult)
                nc.vector.tensor_tensor(out=var[:], in0=var[:], in1=musq[:],
                                        op=ALU.subtract)
                nc.vector.tensor_scalar(out=var[:], in0=var[:], scalar1=1e-12,
                                        scalar2=None, op0=ALU.max)
                sig = st("sig")
                nc.scalar.activation(sig[:], var[:], AF.Sqrt)
                sigz = st("sigz")
                nc.vector.tensor_scalar(out=sigz[:], in0=sig[:],
                                        scalar1=col(C_Z), scalar2=None,
                                        op0=ALU.mult)
                t0hi = st("t0hi")
                nc.vector.tensor_tensor(out=t0hi[:], in0=mu[:], in1=sigz[:],
                                        op=ALU.add)
                t0lo = st("t0lo")
                nc.vector.tensor_tensor(out=t0lo[:], in0=mu[:], in1=sigz[:],
                                        op=ALU.subtract)
                negt0lo = st("negt0lo")
                nc.vector.tensor_scalar(out=negt0lo[:], in0=t0lo[:],
                                        scalar1=-1.0, scalar2=None,
                                        op0=ALU.mult)

                # counts at t0: c_gt on DVE (is_gt), c_lt on ACT (Sign)
                cgt = st("cgt")
                sgn = st("sgn")
                for ft in range(HT):
                    scr = scrtile("d")
                    nc.vector.tensor_scalar(
                        out=scr[:, 0:fdi], in0=esb[ft][:],
                        scalar1=t0hi[:, ft:ft + 1],
                        scalar2=0.0, op0=ALU.is_gt, op1=ALU.add,
                        accum_out=cgt[:, ft:ft + 1])
                    scr2 = scrtile("a")
                    nc.scalar.activation(
                        scr2[:, 0:fdi], esb[ft][:], AF.Sign,
                        bias=negt0lo[:, ft:ft + 1], scale=1.0,
                        accum_out=sgn[:, ft:ft + 1])
                # c_lt_all = (FD - sgn_sum)/2
                clt = st("clt")
                nc.vector.tensor_scalar(out=clt[:], in0=sgn[:],
                                        scalar1=-0.5, scalar2=fd / 2.0,
                                        op0=ALU.mult, op1=ALU.add)

                def pad_count_fix(cnt, thr, op):
                    tmp = st("tmpa")
                    nc.vector.tensor_tensor(out=tmp[:], in0=pdev[:], in1=thr[:],
                                            op=op)
                    nc.vector.tensor_scalar(out=tmp[:], in0=tmp[:],
                                            scalar1=col(C_PADC), scalar2=None,
                                            op0=ALU.mult)
                    nc.vector.tensor_tensor(out=cnt[:], in0=cnt[:], in1=tmp[:],
                                            op=ALU.subtract)
                pad_count_fix(cgt, t0hi, ALU.is_gt)
                pad_count_fix(clt, t0lo, ALU.is_lt)

                # Newton step: t1 = t0 +/- (c - k) * sig / A
                def newton(tout, t0_, cnt, sgn_):
                    d = st("tmpb")
                    nc.vector.tensor_scalar(out=d[:], in0=cnt[:],
                                            scalar1=col(C_K), scalar2=None,
                                            op0=ALU.subtract)
                    nc.vector.tensor_tensor(out=d[:], in0=d[:], in1=sig[:],
                                            op=ALU.mult)
                    nc.vector.tensor_scalar(out=d[:], in0=d[:],
                                            scalar1=col(C_INVA), scalar2=None,
                                            op0=ALU.mult)
                    nc.vector.tensor_tensor(out=tout[:], in0=t0_[:], in1=d[:],
                                            op=(ALU.add if sgn_ > 0
                                                else ALU.subtract))
                t1hi = st("t1hi")
                newton(t1hi, t0hi, cgt, +1)
                t1lo = st("t1lo")
                newton(t1lo, t0lo, clt, -1)

                # S_hi on ACT: sum relu(x - t1hi); S_lo on DVE via sum min(x, t1lo)
                negthi = st("negthi")
                nc.vector.tensor_scalar(out=negthi[:], in0=t1hi[:],
                                        scalar1=-1.0, scalar2=None,
                                        op0=ALU.mult)
                shi = st("shi")
                smin = st("smin")
                for ft in range(HT):
                    scr = scrtile("a")
                    nc.scalar.activation(scr[:, 0:fdi], esb[ft][:], AF.Relu,
                                         bias=negthi[:, ft:ft + 1], scale=1.0,
                                         accum_out=shi[:, ft:ft + 1])
                    scr2 = scrtile("d")
                    nc.vector.tensor_scalar(
                        out=scr2[:, 0:fdi], in0=esb[ft][:],
                        scalar1=t1lo[:, ft:ft + 1], scalar2=0.0,
                        op0=ALU.min, op1=ALU.add,
                        accum_out=smin[:, ft:ft + 1])

                # pad fixes:
                #   shi -= padc * relu(pdev - t1hi)
                #   smin -= padc * min(pdev, t1lo)
                tmp = st("tmpa")
                nc.vector.tensor_tensor(out=tmp[:], in0=pdev[:], in1=t1hi[:],
                                        op=ALU.subtract)
                nc.vector.tensor_scalar(out=tmp[:], in0=tmp[:], scalar1=0.0,
                                        scalar2=None, op0=ALU.max)
                nc.vector.tensor_scalar(out=tmp[:], in0=tmp[:],
                                        scalar1=col(C_PADC), scalar2=None,
                                        op0=ALU.mult)
                nc.vector.tensor_tensor(out=shi[:], in0=shi[:], in1=tmp[:],
                                        op=ALU.subtract)
                tmp2 = st("tmpa")
                nc.vector.tensor_tensor(out=tmp2[:], in0=pdev[:], in1=t1lo[:],
                                        op=ALU.min)
                nc.vector.tensor_scalar(out=tmp2[:], in0=tmp2[:],
                                        scalar1=col(C_PADC), scalar2=None,
                                        op0=ALU.mult)
                nc.vector.tensor_tensor(out=smin[:], in0=smin[:], in1=tmp2[:],
                                        op=ALU.subtract)
                # S_lo = L * t1lo - smin_valid
                slo = st("slo")
                nc.vector.tensor_scalar(out=slo[:], in0=t1lo[:],
                                        scalar1=col(C_L), scalar2=None,
                                        op0=ALU.mult)
                nc.vector.tensor_tensor(out=slo[:], in0=slo[:], in1=smin[:],
                                        op=ALU.subtract)

                # assemble: trimmed = (sumx - top - bot) / denom
                top = st("top")
                nc.vector.tensor_scalar(out=top[:], in0=t1hi[:],
                                        scalar1=col(C_K), scalar2=None,
                                        op0=ALU.mult)
                nc.vector.tensor_tensor(out=top[:], in0=top[:], in1=shi[:],
                                        op=ALU.add)
                bot = st("bot")
                nc.vector.tensor_scalar(out=bot[:], in0=t1lo[:],
                                        scalar1=col(C_K), scalar2=None,
                                        op0=ALU.mult)
                nc.vector.tensor_tensor(out=bot[:], in0=bot[:], in1=slo[:],
                                        op=ALU.subtract)
                trm = st("trm")
                nc.vector.tensor_tensor(out=trm[:], in0=sumx[:], in1=top[:],
                                        op=ALU.subtract)
                nc.vector.tensor_tensor(out=trm[:], in0=trm[:], in1=bot[:],
                                        op=ALU.subtract)
                nc.vector.tensor_scalar(
                    out=trimmed[:, s * HT:(s + 1) * HT], in0=trm[:],
                    scalar1=col(C_INVDEN), scalar2=None, op0=ALU.mult)

            # ---- software-pipelined emission --------------------------------
            pend = {}
            for s in range(SPC + 1):
                if s < SPC:
                    pend[s] = emit_encode(s)
                if s >= 1:
                    emit_select(s - 1, *pend.pop(s - 1))

            # ---- decode (batched over samples) ------------------------------
            trT = trimmed.rearrange("p (s f) -> p f s", f=HT)
            for ht3 in range(HT):
                w3c = pws.tile([128, HT * 128], F32R, tag="w3c",
                               name=f"w3c{ht3}")
                nc.sync.dma_start(
                    out=w3c.rearrange("p (t q) -> p t q", t=HT),
                    in_=W3[:, ht3 * 128:(ht3 + 1) * 128]
                        .rearrange("(t p) h -> p t h", p=128))
                dp = ps_tp.tile([128, SPC], F32, tag="tp", name=f"dp{ht3}")
                for kt in range(HT):
                    nc.tensor.matmul(
                        dp[:], w3c[:, kt * 128:(kt + 1) * 128], trT[:, kt, :],
                        start=(kt == 0), stop=(kt == HT - 1))
                nc.vector.tensor_scalar(
                    out=h3sb[:, ht3 * SPC:(ht3 + 1) * SPC], in0=dp[:],
                    scalar1=b3t[:, ht3:ht3 + 1], scalar2=0.0,
                    op0=ALU.add, op1=ALU.max)
            op_ = ps_h.tile([NOUT, SPC], F32, tag="hp", name="op_")
            for ht3 in range(HT):
                nc.tensor.matmul(
                    op_[:], w4[:, ht3 * NOUT:(ht3 + 1) * NOUT],
                    h3sb[:, ht3 * SPC:(ht3 + 1) * SPC],
                    start=(ht3 == 0), stop=(ht3 == HT - 1))
            outsb = pc.tile([NOUT, SPC], F32, tag="outsb")
            nc.scalar.activation(outsb[:], op_[:], AF.Identity, bias=b4t[:],
                                 scale=1.0)
            nc.sync.dma_start(out=Y[:], in_=outsb[:])

    nc.compile()
    _BUILD_CACHE[fds] = nc
    return nc


def kernel(**inputs):
    X = np.ascontiguousarray(np.asarray(inputs["X"], dtype=np.float32))
    mask = np.asarray(inputs["mask"], dtype=np.float32)
    W1 = np.ascontiguousarray(np.asarray(inputs["W1"], dtype=np.float32))
    b1 = np.asarray(inputs["b1"], dtype=np.float32)
    W2 = np.ascontiguousarray(np.asarray(inputs["W2"], dtype=np.float32))
    b2 = np.asarray(inputs["b2"], dtype=np.float32)
    W3 = np.ascontiguousarray(np.asarray(inputs["W3"], dtype=np.float32))
    b3 = np.asarray(inputs["b3"], dtype=np.float32)
    W4 = np.ascontiguousarray(np.asarray(inputs["W4"], dtype=np.float32))
    b4 = np.asarray(inputs["b4"], dtype=np.float32).reshape(-1)

    L = mask.sum(axis=1).astype(np.int64)                  # [B]
    k = np.floor(L.astype(np.float64) * TRIM_RATIO).astype(np.int64)
    Xm = X * mask[:, :, None]                              # zero pad rows

    # sorted round-robin slot assignment: slot s of core c gets sample
    # order[s * NCORES + c]; slot FD = max L in slot rounded up to 128.
    order = np.argsort(-L, kind="stable")
    fds = []
    for s in range(SPC):
        grp = order[s * NCORES:(s + 1) * NCORES]
        fds.append(int(min(N, -(-int(L[grp].max()) // 128) * 128)))
    fds = tuple(fds)

    CONST = np.zeros((NCORES, SPC, 128, NCC), np.float32)
    Xc = np.zeros((NCORES, SPC, N, D_IN), np.float32)
    for s in range(SPC):
        for c in range(NCORES):
            bidx = int(order[s * NCORES + c])
            Lb, kb = float(L[bidx]), float(k[bidx])
            CONST[c, s, :, C_INVL] = 1.0 / Lb
            z = _norm_ppf(1.0 - kb / Lb) if kb > 0 else 3.0
            CONST[c, s, :, C_Z] = z
            CONST[c, s, :, C_K] = kb
            phi = np.exp(-0.5 * z * z) / np.sqrt(2 * np.pi)
            CONST[c, s, :, C_INVA] = 1.0 / (Lb * phi)
            CONST[c, s, :, C_INVDEN] = 1.0 / (Lb - 2.0 * kb)
            CONST[c, s, :, C_PADC] = float(fds[s] - L[bidx])
            CONST[c, s, :, C_L] = Lb
            Xc[c, s] = Xm[bidx]

    nc = _build_program(fds)
    shared = {
        "W1": W1, "W2": W2, "W3": W3, "W4": W4,
        "B1": np.ascontiguousarray(b1.reshape(HT, 128).T),
        "B2": np.ascontiguousarray(b2.reshape(HT, 128).T),
        "B3": np.ascontiguousarray(b3.reshape(HT, 128).T),
        "B4": np.ascontiguousarray(b4.reshape(NOUT, 1)),
    }
    in_maps = []
    for c in range(NCORES):
        m = dict(shared)
        m["X"] = np.ascontiguousarray(Xc[c])
        m["CONST"] = np.ascontiguousarray(CONST[c])
        in_maps.append(m)

    res = run_bass_kernel_spmd(nc, in_maps, list(range(NCORES)), trace=_TRACE)
    _BUILD_CACHE["last_res"] = res
    out = np.zeros((B, NOUT), np.float32)
    for s in range(SPC):
        for c in range(NCORES):
            out[int(order[s * NCORES + c]), :] = res.results[c]["Y"][:, s]
    return out
